# revision 3
# baseline (speedup 1.0000x reference)
"""AdaptiveSoftmax forward on 8 TRN2 NeuronCores.

Strategy: data-parallel over the 4096 tokens (512/core), no collectives.
 - head:  [512,1024] @ [1024,10002] per core, dense.
 - tails: moe-style routing — band rows are compacted on host, each core
   computes proj+out matmuls only for its ~205 band tokens (padded to a
   static NB), and the host scatters results into the zero-filled dense
   output. Out-of-band rows of the reference are exactly zero.
 - all device matmuls in bf16 (inputs pre-transposed/cast on host),
   fp32 PSUM accumulation, bf16 outputs upcast on host.
 - the tiny int32 retarget vectors are pure indexing; computed on host.
"""

import os

import numpy as np
import ml_dtypes

CUTOFF = (10000, 30000, 50000)
D = 1024
VH = CUTOFF[0] + 2          # 10002
VT = CUTOFF[1] - CUTOFF[0]  # 20000 (both tails)
D0 = 256
D1 = 64
N_CORES = 8
P = 128

BF16 = ml_dtypes.bfloat16

_graph_cache = {}


def _build_graph(T, NB0, NB1):
    """Build the SPMD bacc graph for per-core token count T and padded
    band sizes NB0/NB1 (all multiples of 128)."""
    import concourse.bacc as bacc
    import concourse.tile as tile
    import concourse.mybir as mybir

    f32 = mybir.dt.float32
    bf16 = mybir.dt.bfloat16

    KT = D // P          # 8 contraction tiles of 128
    MT = T // P          # token tiles for the head
    M0 = NB0 // P
    M1 = NB1 // P

    nc = bacc.Bacc("TRN2", target_bir_lowering=False, debug=False,
                   num_devices=N_CORES)

    hT_e = nc.dram_tensor("hT", (D, T), bf16, kind="ExternalInput")
    h0T_e = nc.dram_tensor("h0T", (D, NB0), bf16, kind="ExternalInput")
    h1T_e = nc.dram_tensor("h1T", (D, NB1), bf16, kind="ExternalInput")
    hw_e = nc.dram_tensor("head_wT", (D, VH), bf16, kind="ExternalInput")
    p0w_e = nc.dram_tensor("proj0_wT", (D, D0), bf16, kind="ExternalInput")
    p1w_e = nc.dram_tensor("proj1_wT", (D, D1), bf16, kind="ExternalInput")
    o0w_e = nc.dram_tensor("out0_wT", (D0, VT), bf16, kind="ExternalInput")
    o1w_e = nc.dram_tensor("out1_wT", (D1, VT), bf16, kind="ExternalInput")
    ho_e = nc.dram_tensor("head_o", (T, VH), bf16, kind="ExternalOutput")
    t0_e = nc.dram_tensor("t0_o", (NB0, VT), bf16, kind="ExternalOutput")
    t1_e = nc.dram_tensor("t1_o", (NB1, VT), bf16, kind="ExternalOutput")

    def col_tiles(total, sz):
        out, c = [], 0
        while c < total:
            out.append((c, min(sz, total - c)))
            c += sz
        return out

    with tile.TileContext(nc) as tc:
        with (
            tc.tile_pool(name="res", bufs=1) as rpool,
            tc.tile_pool(name="wstrip", bufs=4) as wpool,
            tc.tile_pool(name="ostage", bufs=8) as opool,
            tc.tile_pool(name="psmain", bufs=6, space="PSUM") as psm,
            tc.tile_pool(name="psproj", bufs=1, space="PSUM") as psp,
        ):
            # ---- resident activations / small weights ----
            hT = rpool.tile([P, KT * T], bf16, tag="hT")
            for k in range(KT):
                nc.sync.dma_start(hT[:, k * T:(k + 1) * T],
                                  hT_e.ap()[k * P:(k + 1) * P, :])
            h0T = rpool.tile([P, KT * NB0], bf16, tag="h0T")
            for k in range(KT):
                nc.sync.dma_start(h0T[:, k * NB0:(k + 1) * NB0],
                                  h0T_e.ap()[k * P:(k + 1) * P, :])
            h1T = rpool.tile([P, KT * NB1], bf16, tag="h1T")
            for k in range(KT):
                nc.sync.dma_start(h1T[:, k * NB1:(k + 1) * NB1],
                                  h1T_e.ap()[k * P:(k + 1) * P, :])
            p0w = rpool.tile([P, KT * D0], bf16, tag="p0w")
            for k in range(KT):
                nc.sync.dma_start(p0w[:, k * D0:(k + 1) * D0],
                                  p0w_e.ap()[k * P:(k + 1) * P, :])
            p1w = rpool.tile([P, KT * D1], bf16, tag="p1w")
            for k in range(KT):
                nc.sync.dma_start(p1w[:, k * D1:(k + 1) * D1],
                                  p1w_e.ap()[k * P:(k + 1) * P, :])

            # ---- proj0: p0T[d0, NB0] = proj0_w @ h0  (2 partition tiles) ----
            p0T = rpool.tile([P, 2 * NB0], bf16, tag="p0T")
            for mp in range(2):
                ps = psp.tile([P, NB0], f32, tag="ps0")
                for k in range(KT):
                    nc.tensor.matmul(
                        ps[:],
                        p0w[:, k * D0 + mp * P: k * D0 + (mp + 1) * P],
                        h0T[:, k * NB0:(k + 1) * NB0],
                        start=(k == 0), stop=(k == KT - 1),
                    )
                nc.any.tensor_copy(p0T[:, mp * NB0:(mp + 1) * NB0], ps[:])

            # ---- proj1: p1T[d1, NB1] ----
            p1T = rpool.tile([D1, NB1], bf16, tag="p1T")
            ps = psp.tile([D1, NB1], f32, tag="ps1")
            for k in range(KT):
                nc.tensor.matmul(
                    ps[:],
                    p1w[:, k * D1:(k + 1) * D1],
                    h1T[:, k * NB1:(k + 1) * NB1],
                    start=(k == 0), stop=(k == KT - 1),
                )
            nc.any.tensor_copy(p1T[:], ps[:])

            # ---- head: ho[T, VH] ----
            for col, nsz in col_tiles(VH, 512):
                strips = wpool.tile([P, KT * nsz], bf16, tag="wstrip")
                for k in range(KT):
                    nc.sync.dma_start(strips[:, k * nsz:(k + 1) * nsz],
                                      hw_e.ap()[k * P:(k + 1) * P,
                                                col:col + nsz])
                for m in range(MT):
                    ps = psm.tile([P, nsz], f32, tag="ps")
                    for k in range(KT):
                        nc.tensor.matmul(
                            ps[:],
                            hT[:, k * T + m * P: k * T + (m + 1) * P],
                            strips[:, k * nsz:(k + 1) * nsz],
                            start=(k == 0), stop=(k == KT - 1),
                        )
                    ot = opool.tile([P, nsz], bf16, tag="o")
                    nc.any.tensor_copy(ot[:], ps[:])
                    nc.sync.dma_start(
                        ho_e.ap()[m * P:(m + 1) * P, col:col + nsz], ot[:])

            # ---- tail0: t0[NB0, VT] = p0.T @ out0_wT ----
            for col, nsz in col_tiles(VT, 512):
                s0 = wpool.tile([P, 2 * nsz], bf16, tag="wstrip")
                for kp in range(2):
                    nc.sync.dma_start(s0[:, kp * nsz:(kp + 1) * nsz],
                                      o0w_e.ap()[kp * P:(kp + 1) * P,
                                                 col:col + nsz])
                for m in range(M0):
                    ps = psm.tile([P, nsz], f32, tag="ps")
                    for kp in range(2):
                        nc.tensor.matmul(
                            ps[:],
                            p0T[:, kp * NB0 + m * P: kp * NB0 + (m + 1) * P],
                            s0[:, kp * nsz:(kp + 1) * nsz],
                            start=(kp == 0), stop=(kp == 1),
                        )
                    ot = opool.tile([P, nsz], bf16, tag="o")
                    nc.any.tensor_copy(ot[:], ps[:])
                    nc.sync.dma_start(
                        t0_e.ap()[m * P:(m + 1) * P, col:col + nsz], ot[:])

            # ---- tail1: t1[NB1, VT] = p1.T @ out1_wT (single K=64) ----
            for col, nsz in col_tiles(VT, 512):
                s1 = wpool.tile([D1, nsz], bf16, tag="wstrip")
                nc.sync.dma_start(s1[:], o1w_e.ap()[:, col:col + nsz])
                for m in range(M1):
                    ps = psm.tile([P, nsz], f32, tag="ps")
                    nc.tensor.matmul(
                        ps[:],
                        p1T[:, m * P:(m + 1) * P],
                        s1[:],
                        start=True, stop=True,
                    )
                    ot = opool.tile([P, nsz], bf16, tag="o")
                    nc.any.tensor_copy(ot[:], ps[:])
                    nc.sync.dma_start(
                        t1_e.ap()[m * P:(m + 1) * P, col:col + nsz], ot[:])

    nc.compile()
    return nc


def _pad128(n):
    return max(P, ((n + P - 1) // P) * P)


def _install_ntff_hook():
    """Register the NTFF profile hook the agent image's antenv lacks, so
    run_bass_kernel_spmd(trace=True) can report exec_time_ns under axon."""
    import sys
    import types
    if 'antenv.axon_hooks' in sys.modules:
        return
    mod = types.ModuleType('antenv.axon_hooks')
    mod._hook = None
    mod.set_axon_ntff_profile_hook = lambda h: setattr(mod, '_hook', h)
    mod.get_axon_ntff_profile_hook = lambda: mod._hook
    sys.modules['antenv.axon_hooks'] = mod
    import antenv
    antenv.axon_hooks = mod
    from trn_agent_boot.trn_boot import _ntff_profile_via_ctypes
    mod._hook = _ntff_profile_via_ctypes('/opt/axon/libaxon_pjrt.so')
    import concourse.bass_utils as bu
    bu.upload_artifacts = lambda tmpdir: f"local:{tmpdir}"


def _run_spmd(nc, in_maps, profile):
    from concourse.bass_utils import run_bass_kernel_spmd
    kwargs = {}
    if profile:
        kwargs["trace"] = True
    return run_bass_kernel_spmd(nc, in_maps, core_ids=list(range(N_CORES)),
                                **kwargs)


def kernel(input, target, head_w, proj0_w, out0_w, proj1_w, out1_w):
    c0, c1, c2 = CUTOFF
    h = np.ascontiguousarray(np.asarray(input, dtype=np.float32)).reshape(-1, D)
    t = np.asarray(target, dtype=np.int32).reshape(-1)
    N = h.shape[0]
    assert N % N_CORES == 0
    T = N // N_CORES

    m0 = (t >= c0) & (t < c1)
    m1 = (t >= c1) & (t < c2)

    # int32 retarget outputs (pure indexing, negligible)
    new_t_head = np.where(m0, c0, np.where(m1, c0 + 1, t)).astype(np.int32)
    new_t0 = np.where(m0, t - c0, 0).astype(np.int32)
    new_t1 = np.where(m1, t - c1, 0).astype(np.int32)

    # per-core routing of band rows
    idx0 = [np.nonzero(m0[c * T:(c + 1) * T])[0] for c in range(N_CORES)]
    idx1 = [np.nonzero(m1[c * T:(c + 1) * T])[0] for c in range(N_CORES)]
    NB0 = _pad128(max(len(i) for i in idx0))
    NB1 = _pad128(max(len(i) for i in idx1))

    key = (T, NB0, NB1)
    if key not in _graph_cache:
        _graph_cache[key] = _build_graph(T, NB0, NB1)
    nc = _graph_cache[key]

    def bft(a):  # bf16 transposed copy
        return np.ascontiguousarray(a.T).astype(BF16)

    w_shared = {
        "head_wT": bft(np.asarray(head_w, dtype=np.float32)),
        "proj0_wT": bft(np.asarray(proj0_w, dtype=np.float32)),
        "proj1_wT": bft(np.asarray(proj1_w, dtype=np.float32)),
        "out0_wT": bft(np.asarray(out0_w, dtype=np.float32)),
        "out1_wT": bft(np.asarray(out1_w, dtype=np.float32)),
    }

    in_maps = []
    for c in range(N_CORES):
        hc = h[c * T:(c + 1) * T]
        h0 = np.zeros((NB0, D), np.float32)
        h0[:len(idx0[c])] = hc[idx0[c]]
        h1 = np.zeros((NB1, D), np.float32)
        h1[:len(idx1[c])] = hc[idx1[c]]
        in_maps.append({
            "hT": bft(hc),
            "h0T": bft(h0),
            "h1T": bft(h1),
            **w_shared,
        })

    profile = bool(os.environ.get("BASS_KERNEL_PROFILE"))
    if profile:
        try:
            _install_ntff_hook()
        except Exception:
            pass
    res = _run_spmd(nc, in_maps, profile)
    if profile and res.exec_time_ns is not None:
        print(f"HW exec time: {res.exec_time_ns} ns")
        kernel.last_exec_time_ns = res.exec_time_ns

    head_out = np.empty((N, VH), np.float32)
    tail0 = np.zeros((N, VT), np.float32)
    tail1 = np.zeros((N, VT), np.float32)
    for c in range(N_CORES):
        r = res.results[c]
        head_out[c * T:(c + 1) * T] = r["head_o"].astype(np.float32)
        if len(idx0[c]):
            tail0[c * T + idx0[c]] = r["t0_o"][:len(idx0[c])].astype(np.float32)
        if len(idx1[c]):
            tail1[c * T + idx1[c]] = r["t1_o"][:len(idx1[c])].astype(np.float32)

    return (head_out, tail0, tail1, new_t_head, new_t0, new_t1)


# revision 4
# speedup vs baseline: 1.7118x; 1.7118x over previous
"""AdaptiveSoftmax forward on 8 TRN2 NeuronCores.

Strategy: data-parallel over the 4096 tokens (512/core), no collectives.
 - head:  [512,1024] @ [1024,10002] per core, dense.
 - tails: moe-style routing — band rows are compacted on host, each core
   computes proj+out matmuls only for its ~205 band tokens (padded to a
   static NB), and the host scatters results into the zero-filled dense
   output. Out-of-band rows of the reference are exactly zero.
 - all device matmuls in bf16 (inputs pre-transposed/cast/interleaved on
   host so every SBUF strip is a single large contiguous DMA), fp32 PSUM
   accumulation, bf16 outputs upcast on host.
 - the tiny int32 retarget vectors are pure indexing; computed on host.
"""

import os

import numpy as np
import ml_dtypes

CUTOFF = (10000, 30000, 50000)
D = 1024
VH = CUTOFF[0] + 2          # 10002
VT = CUTOFF[1] - CUTOFF[0]  # 20000 (both tails)
D0 = 256
D1 = 64
N_CORES = 8
P = 128
KT = D // P                 # 8 contraction tiles over D
W_HEAD = 1024               # output column group width (head)
W_TAIL = 2048               # output column group width (tails)

BF16 = ml_dtypes.bfloat16

_graph_cache = {}


def _groups(total, tile_w, group_tiles):
    """[(col0, [tile widths]), ...] covering `total` columns."""
    tiles, c = [], 0
    while c < total:
        tiles.append(min(tile_w, total - c))
        c += tiles[-1]
    out = []
    for i in range(0, len(tiles), group_tiles):
        chunk = tiles[i:i + group_tiles]
        out.append((sum(t for g in out for t in g[1]), chunk))
    return out


HEAD_GROUPS = _groups(VH, 512, W_HEAD // 512)
TAIL_GROUPS = _groups(VT, 512, W_TAIL // 512)


def _build_graph(T, NB0, NB1):
    """SPMD bacc graph for per-core token count T and padded band sizes
    NB0/NB1 (multiples of 128)."""
    import concourse.bacc as bacc
    import concourse.tile as tile
    import concourse.mybir as mybir

    f32 = mybir.dt.float32
    bf16 = mybir.dt.bfloat16

    MT = T // P
    M0 = NB0 // P
    M1 = NB1 // P

    nc = bacc.Bacc("TRN2", target_bir_lowering=False, debug=False,
                   num_devices=N_CORES)

    hT_e = nc.dram_tensor("hT", (P, KT * T), bf16, kind="ExternalInput")
    h0T_e = nc.dram_tensor("h0T", (P, KT * NB0), bf16, kind="ExternalInput")
    h1T_e = nc.dram_tensor("h1T", (P, KT * NB1), bf16, kind="ExternalInput")
    hw_e = nc.dram_tensor("head_wI", (P, KT * VH), bf16, kind="ExternalInput")
    p0w_e = nc.dram_tensor("proj0_wI", (P, KT * D0), bf16, kind="ExternalInput")
    p1w_e = nc.dram_tensor("proj1_wI", (P, KT * D1), bf16, kind="ExternalInput")
    o0w_e = nc.dram_tensor("out0_wI", (P, 2 * VT), bf16, kind="ExternalInput")
    o1w_e = nc.dram_tensor("out1_wT", (D1, VT), bf16, kind="ExternalInput")
    ho_e = nc.dram_tensor("head_o", (T, VH), bf16, kind="ExternalOutput")
    t0_e = nc.dram_tensor("t0_o", (NB0, VT), bf16, kind="ExternalOutput")
    t1_e = nc.dram_tensor("t1_o", (NB1, VT), bf16, kind="ExternalOutput")

    copy_ctr = [0]

    with tile.TileContext(nc) as tc:
        with (
            tc.tile_pool(name="res", bufs=1) as rpool,
            tc.tile_pool(name="wstrip", bufs=4) as wpool,
            tc.tile_pool(name="ostage", bufs=6) as opool,
            tc.tile_pool(name="psmain", bufs=6, space="PSUM") as psm,
            tc.tile_pool(name="psproj", bufs=1, space="PSUM") as psp,
        ):
            def evict(dst, src):
                if copy_ctr[0] % 2 == 0:
                    nc.scalar.copy(dst, src)
                else:
                    nc.vector.tensor_copy(dst, src)
                copy_ctr[0] += 1

            # ---- resident activations / small weights (1 DMA each) ----
            hT = rpool.tile([P, KT * T], bf16, tag="hT")
            nc.sync.dma_start(hT[:], hT_e.ap()[:])
            h0T = rpool.tile([P, KT * NB0], bf16, tag="h0T")
            nc.sync.dma_start(h0T[:], h0T_e.ap()[:])
            h1T = rpool.tile([P, KT * NB1], bf16, tag="h1T")
            nc.sync.dma_start(h1T[:], h1T_e.ap()[:])
            p0w = rpool.tile([P, KT * D0], bf16, tag="p0w")
            nc.sync.dma_start(p0w[:], p0w_e.ap()[:])
            p1w = rpool.tile([P, KT * D1], bf16, tag="p1w")
            nc.sync.dma_start(p1w[:], p1w_e.ap()[:])

            # ---- proj0: p0T[d0, NB0] (2 partition tiles) ----
            p0T = rpool.tile([P, 2 * NB0], bf16, tag="p0T")
            for mp in range(2):
                ps = psp.tile([P, NB0], f32, tag="ps0")
                for k in range(KT):
                    nc.tensor.matmul(
                        ps[:],
                        p0w[:, k * D0 + mp * P: k * D0 + (mp + 1) * P],
                        h0T[:, k * NB0:(k + 1) * NB0],
                        start=(k == 0), stop=(k == KT - 1),
                    )
                evict(p0T[:, mp * NB0:(mp + 1) * NB0], ps[:])

            # ---- proj1: p1T[d1, NB1] ----
            p1T = rpool.tile([D1, NB1], bf16, tag="p1T")
            ps = psp.tile([D1, NB1], f32, tag="ps1")
            for k in range(KT):
                nc.tensor.matmul(
                    ps[:],
                    p1w[:, k * D1:(k + 1) * D1],
                    h1T[:, k * NB1:(k + 1) * NB1],
                    start=(k == 0), stop=(k == KT - 1),
                )
            evict(p1T[:], ps[:])

            # ---- head: ho[T, VH] ----
            for c0, tiles in HEAD_GROUPS:
                W = sum(tiles)
                strips = wpool.tile([P, KT * W], bf16, tag="ws")
                nc.sync.dma_start(strips[:],
                                  hw_e.ap()[:, KT * c0: KT * (c0 + W)])
                for m in range(MT):
                    ot = opool.tile([P, W], bf16, tag="o")
                    j = 0
                    for nsz in tiles:
                        ps = psm.tile([P, nsz], f32, tag="ps")
                        for k in range(KT):
                            nc.tensor.matmul(
                                ps[:],
                                hT[:, k * T + m * P: k * T + (m + 1) * P],
                                strips[:, k * W + j: k * W + j + nsz],
                                start=(k == 0), stop=(k == KT - 1),
                            )
                        evict(ot[:, j:j + nsz], ps[:])
                        j += nsz
                    nc.sync.dma_start(
                        ho_e.ap()[m * P:(m + 1) * P, c0:c0 + W], ot[:])

            # ---- tail0: t0[NB0, VT] ----
            for c0, tiles in TAIL_GROUPS:
                W = sum(tiles)
                s0 = wpool.tile([P, 2 * W], bf16, tag="ws")
                nc.sync.dma_start(s0[:],
                                  o0w_e.ap()[:, 2 * c0: 2 * (c0 + W)])
                for m in range(M0):
                    ot = opool.tile([P, W], bf16, tag="o")
                    j = 0
                    for nsz in tiles:
                        ps = psm.tile([P, nsz], f32, tag="ps")
                        for kp in range(2):
                            nc.tensor.matmul(
                                ps[:],
                                p0T[:, kp * NB0 + m * P: kp * NB0 + (m + 1) * P],
                                s0[:, kp * W + j: kp * W + j + nsz],
                                start=(kp == 0), stop=(kp == 1),
                            )
                        evict(ot[:, j:j + nsz], ps[:])
                        j += nsz
                    nc.sync.dma_start(
                        t0_e.ap()[m * P:(m + 1) * P, c0:c0 + W], ot[:])

            # ---- tail1: t1[NB1, VT] (single K=64) ----
            for c0, tiles in TAIL_GROUPS:
                W = sum(tiles)
                s1 = wpool.tile([D1, W], bf16, tag="ws1")
                nc.sync.dma_start(s1[:], o1w_e.ap()[:, c0:c0 + W])
                for m in range(M1):
                    ot = opool.tile([P, W], bf16, tag="o")
                    j = 0
                    for nsz in tiles:
                        ps = psm.tile([P, nsz], f32, tag="ps")
                        nc.tensor.matmul(
                            ps[:],
                            p1T[:, m * P:(m + 1) * P],
                            s1[:, j:j + nsz],
                            start=True, stop=True,
                        )
                        evict(ot[:, j:j + nsz], ps[:])
                        j += nsz
                    nc.sync.dma_start(
                        t1_e.ap()[m * P:(m + 1) * P, c0:c0 + W], ot[:])

    nc.compile()
    return nc


def _pad128(n):
    return max(P, ((n + P - 1) // P) * P)


def _install_ntff_hook():
    """Register the NTFF profile hook the agent image's antenv lacks, so
    run_bass_kernel_spmd(trace=True) can report exec_time_ns under axon."""
    import sys
    import types
    if 'antenv.axon_hooks' in sys.modules:
        return
    mod = types.ModuleType('antenv.axon_hooks')
    mod._hook = None
    mod.set_axon_ntff_profile_hook = lambda h: setattr(mod, '_hook', h)
    mod.get_axon_ntff_profile_hook = lambda: mod._hook
    sys.modules['antenv.axon_hooks'] = mod
    import antenv
    antenv.axon_hooks = mod
    from trn_agent_boot.trn_boot import _ntff_profile_via_ctypes
    mod._hook = _ntff_profile_via_ctypes('/opt/axon/libaxon_pjrt.so')
    import concourse.bass_utils as bu
    bu.upload_artifacts = lambda tmpdir: f"local:{tmpdir}"


def _run_spmd(nc, in_maps, profile):
    from concourse.bass_utils import run_bass_kernel_spmd
    kwargs = {}
    if profile:
        kwargs["trace"] = True
    return run_bass_kernel_spmd(nc, in_maps, core_ids=list(range(N_CORES)),
                                **kwargs)


def _interleave_k(a_t, kt):
    """[kt*P, F] -> [P, kt*F] with X[p, k*F + j] = a_t[k*P + p, j]."""
    ktp, F = a_t.shape
    assert ktp == kt * P
    return np.ascontiguousarray(
        a_t.reshape(kt, P, F).transpose(1, 0, 2).reshape(P, kt * F))


def _interleave_groups(a_t, kt, groups):
    """Group-blocked interleave: for each (c0, tiles) with width W, block
    [P, kt*W] with X[p, k*W + j] = a_t[k*P + p, c0 + j]; blocks concat."""
    A = a_t.reshape(kt, P, a_t.shape[1])
    blocks = []
    for c0, tiles in groups:
        W = sum(tiles)
        blocks.append(A[:, :, c0:c0 + W].transpose(1, 0, 2).reshape(P, kt * W))
    return np.ascontiguousarray(np.concatenate(blocks, axis=1))


def kernel(input, target, head_w, proj0_w, out0_w, proj1_w, out1_w):
    c0_, c1_, c2_ = CUTOFF
    h = np.ascontiguousarray(np.asarray(input, dtype=np.float32)).reshape(-1, D)
    t = np.asarray(target, dtype=np.int32).reshape(-1)
    N = h.shape[0]
    assert N % N_CORES == 0
    T = N // N_CORES

    m0 = (t >= c0_) & (t < c1_)
    m1 = (t >= c1_) & (t < c2_)

    # int32 retarget outputs (pure indexing, negligible)
    new_t_head = np.where(m0, c0_, np.where(m1, c0_ + 1, t)).astype(np.int32)
    new_t0 = np.where(m0, t - c0_, 0).astype(np.int32)
    new_t1 = np.where(m1, t - c1_, 0).astype(np.int32)

    # per-core routing of band rows
    idx0 = [np.nonzero(m0[c * T:(c + 1) * T])[0] for c in range(N_CORES)]
    idx1 = [np.nonzero(m1[c * T:(c + 1) * T])[0] for c in range(N_CORES)]
    NB0 = _pad128(max(len(i) for i in idx0))
    NB1 = _pad128(max(len(i) for i in idx1))

    key = (T, NB0, NB1)
    if key not in _graph_cache:
        _graph_cache[key] = _build_graph(T, NB0, NB1)
    nc = _graph_cache[key]

    bf = lambda a: a.astype(BF16)

    w_shared = {
        "head_wI": bf(_interleave_groups(
            np.asarray(head_w, np.float32).T, KT, HEAD_GROUPS)),
        "proj0_wI": bf(_interleave_k(np.asarray(proj0_w, np.float32).T, KT)),
        "proj1_wI": bf(_interleave_k(np.asarray(proj1_w, np.float32).T, KT)),
        "out0_wI": bf(_interleave_groups(
            np.asarray(out0_w, np.float32).T, 2, TAIL_GROUPS)),
        "out1_wT": bf(np.ascontiguousarray(np.asarray(out1_w, np.float32).T)),
    }

    in_maps = []
    for c in range(N_CORES):
        hc = h[c * T:(c + 1) * T]
        h0 = np.zeros((NB0, D), np.float32)
        h0[:len(idx0[c])] = hc[idx0[c]]
        h1 = np.zeros((NB1, D), np.float32)
        h1[:len(idx1[c])] = hc[idx1[c]]
        in_maps.append({
            "hT": bf(_interleave_k(np.ascontiguousarray(hc.T), KT)),
            "h0T": bf(_interleave_k(np.ascontiguousarray(h0.T), KT)),
            "h1T": bf(_interleave_k(np.ascontiguousarray(h1.T), KT)),
            **w_shared,
        })

    profile = bool(os.environ.get("BASS_KERNEL_PROFILE"))
    if profile:
        try:
            _install_ntff_hook()
        except Exception:
            pass
    res = _run_spmd(nc, in_maps, profile)
    if profile and res.exec_time_ns is not None:
        print(f"HW exec time: {res.exec_time_ns} ns")
        kernel.last_exec_time_ns = res.exec_time_ns

    head_out = np.empty((N, VH), np.float32)
    tail0 = np.zeros((N, VT), np.float32)
    tail1 = np.zeros((N, VT), np.float32)
    for c in range(N_CORES):
        r = res.results[c]
        head_out[c * T:(c + 1) * T] = r["head_o"].astype(np.float32)
        if len(idx0[c]):
            tail0[c * T + idx0[c]] = r["t0_o"][:len(idx0[c])].astype(np.float32)
        if len(idx1[c]):
            tail1[c * T + idx1[c]] = r["t1_o"][:len(idx1[c])].astype(np.float32)

    return (head_out, tail0, tail1, new_t_head, new_t0, new_t1)


# revision 5
# speedup vs baseline: 1.7418x; 1.0175x over previous
"""AdaptiveSoftmax forward on 8 TRN2 NeuronCores.

Strategy: data-parallel over token pairs x 2-way tensor-parallel over
vocab, no collectives.
 - cores (2p, 2p+1) share the same 1024 tokens; the even core computes
   the left vocab half, the odd core the right half (head 5001/5001,
   tails 10000/10000) -> identical SPMD graph, different input data.
 - tails: moe-style routing — band rows are compacted on host, each core
   computes proj+out matmuls only for the pair's ~410 band tokens
   (padded to a static NB), and the host scatters results into the
   zero-filled dense output (out-of-band reference rows are exactly 0).
 - all device matmuls in bf16 (inputs pre-transposed/cast/interleaved on
   host so every SBUF strip is one or two large contiguous DMAs), fp32
   PSUM accumulation, bf16 outputs upcast on host.
 - head/tail0/tail1 column groups are interleaved in emission order so
   the output-DMA rate stays under the HBM roofline while the
   TensorEngine stays busy.
 - the tiny int32 retarget vectors are pure indexing; computed on host.
"""

import os

import numpy as np
import ml_dtypes

CUTOFF = (10000, 30000, 50000)
D = 1024
VH = CUTOFF[0] + 2          # 10002
VT = CUTOFF[1] - CUTOFF[0]  # 20000 (both tails)
VHH = VH // 2               # 5001 per-core head half
VTH = VT // 2               # 10000 per-core tail half
D0 = 256
D1 = 64
N_CORES = 8
N_PAIRS = N_CORES // 2
P = 128
KT = D // P                 # 8 contraction tiles over D

BF16 = ml_dtypes.bfloat16

_graph_cache = {}


def _groups(total, tile_w, group_tiles):
    """[(col0, [tile widths]), ...] covering `total` columns."""
    tiles, c = [], 0
    while c < total:
        tiles.append(min(tile_w, total - c))
        c += tiles[-1]
    out, c = [], 0
    for i in range(0, len(tiles), group_tiles):
        chunk = tiles[i:i + group_tiles]
        out.append((c, chunk))
        c += sum(chunk)
    return out


HEAD_GROUPS = _groups(VHH, 512, 2)   # 5 groups: 4x1024 + 905
TAIL_GROUPS = _groups(VTH, 512, 4)   # 5 groups: 4x2048 + 1808


def _build_graph(TOK, NB0, NB1):
    """SPMD bacc graph. TOK tokens per pair; NB0/NB1 padded band sizes."""
    import concourse.bacc as bacc
    import concourse.tile as tile
    import concourse.mybir as mybir

    f32 = mybir.dt.float32
    bf16 = mybir.dt.bfloat16

    MT = TOK // P
    M0 = NB0 // P
    M1 = NB1 // P

    nc = bacc.Bacc("TRN2", target_bir_lowering=False, debug=False,
                   num_devices=N_CORES)

    hT_e = nc.dram_tensor("hT", (P, KT * TOK), bf16, kind="ExternalInput")
    h0T_e = nc.dram_tensor("h0T", (P, KT * NB0), bf16, kind="ExternalInput")
    h1T_e = nc.dram_tensor("h1T", (P, KT * NB1), bf16, kind="ExternalInput")
    hw_e = nc.dram_tensor("head_wI", (P, KT * VHH), bf16, kind="ExternalInput")
    p0w_e = nc.dram_tensor("proj0_wI", (P, KT * D0), bf16, kind="ExternalInput")
    p1w_e = nc.dram_tensor("proj1_wI", (P, KT * D1), bf16, kind="ExternalInput")
    o0w_e = nc.dram_tensor("out0_wI", (P, 2 * VTH), bf16, kind="ExternalInput")
    o1w_e = nc.dram_tensor("out1_wT", (D1, VTH), bf16, kind="ExternalInput")
    ho_e = nc.dram_tensor("head_o", (TOK, VHH), bf16, kind="ExternalOutput")
    t0_e = nc.dram_tensor("t0_o", (NB0, VTH), bf16, kind="ExternalOutput")
    t1_e = nc.dram_tensor("t1_o", (NB1, VTH), bf16, kind="ExternalOutput")

    copy_ctr = [0]

    with tile.TileContext(nc) as tc:
        with (
            tc.tile_pool(name="res", bufs=1) as rpool,
            tc.tile_pool(name="wstrip", bufs=4) as wpool,
            tc.tile_pool(name="ostage", bufs=6) as opool,
            tc.tile_pool(name="psmain", bufs=6, space="PSUM") as psm,
            tc.tile_pool(name="psproj", bufs=1, space="PSUM") as psp,
        ):
            def evict(dst, src):
                if copy_ctr[0] % 2 == 0:
                    nc.scalar.copy(dst, src)
                else:
                    nc.vector.tensor_copy(dst, src)
                copy_ctr[0] += 1

            def dma_split(dst, src, n):
                F = dst.shape[-1]
                step = -(-F // n)
                j = 0
                while j < F:
                    w = min(step, F - j)
                    nc.sync.dma_start(dst[:, j:j + w], src[:, j:j + w])
                    j += w

            # ---- resident loads; proj inputs first so proj can start ----
            p0w = rpool.tile([P, KT * D0], bf16, tag="p0w")
            nc.sync.dma_start(p0w[:], p0w_e.ap()[:])
            h0T = rpool.tile([P, KT * NB0], bf16, tag="h0T")
            dma_split(h0T[:], h0T_e.ap()[:], 2)
            p1w = rpool.tile([P, KT * D1], bf16, tag="p1w")
            nc.sync.dma_start(p1w[:], p1w_e.ap()[:])
            h1T = rpool.tile([P, KT * NB1], bf16, tag="h1T")
            dma_split(h1T[:], h1T_e.ap()[:], 2)
            hT = rpool.tile([P, KT * TOK], bf16, tag="hT")
            dma_split(hT[:], hT_e.ap()[:], 4)

            # ---- proj0: p0T[d0, NB0] (2 partition tiles) ----
            p0T = rpool.tile([P, 2 * NB0], bf16, tag="p0T")
            for mp in range(2):
                ps = psp.tile([P, NB0], f32, tag="ps0")
                for k in range(KT):
                    nc.tensor.matmul(
                        ps[:],
                        p0w[:, k * D0 + mp * P: k * D0 + (mp + 1) * P],
                        h0T[:, k * NB0:(k + 1) * NB0],
                        start=(k == 0), stop=(k == KT - 1),
                    )
                evict(p0T[:, mp * NB0:(mp + 1) * NB0], ps[:])

            # ---- proj1: p1T[d1, NB1] ----
            p1T = rpool.tile([D1, NB1], bf16, tag="p1T")
            ps = psp.tile([D1, NB1], f32, tag="ps1")
            for k in range(KT):
                nc.tensor.matmul(
                    ps[:],
                    p1w[:, k * D1:(k + 1) * D1],
                    h1T[:, k * NB1:(k + 1) * NB1],
                    start=(k == 0), stop=(k == KT - 1),
                )
            evict(p1T[:], ps[:])

            # ---- one column group of an output matmul ----
            def out_group(c0, tiles, KP, strip_part, w_e, w_il, out_e, M,
                          lhsT_fn, strip_splits):
                W = sum(tiles)
                strip = wpool.tile([strip_part, KP * W], bf16, tag="ws")
                if w_il:
                    dma_split(strip[:], w_e.ap()[:, KP * c0: KP * (c0 + W)],
                              strip_splits)
                else:
                    dma_split(strip[:], w_e.ap()[:, c0:c0 + W], strip_splits)
                for m in range(M):
                    ot = opool.tile([P, W], bf16, tag="o")
                    j = 0
                    for nsz in tiles:
                        ps = psm.tile([P, nsz], f32, tag="ps")
                        for kp in range(KP):
                            nc.tensor.matmul(
                                ps[:],
                                lhsT_fn(kp, m),
                                strip[:, kp * W + j: kp * W + j + nsz],
                                start=(kp == 0), stop=(kp == KP - 1),
                            )
                        evict(ot[:, j:j + nsz], ps[:])
                        j += nsz
                    dma_split(out_e.ap()[m * P:(m + 1) * P, c0:c0 + W],
                              ot[:], 2)

            # ---- interleave head / tail0 / tail1 column groups ----
            n_iter = max(len(HEAD_GROUPS), len(TAIL_GROUPS))
            for i in range(n_iter):
                if i < len(HEAD_GROUPS):
                    c0, tiles = HEAD_GROUPS[i]
                    out_group(
                        c0, tiles, KT, P, hw_e, True, ho_e, MT,
                        lambda k, m: hT[:, k * TOK + m * P: k * TOK + (m + 1) * P],
                        4)
                if i < len(TAIL_GROUPS):
                    c0, tiles = TAIL_GROUPS[i]
                    out_group(
                        c0, tiles, 2, P, o0w_e, True, t0_e, M0,
                        lambda k, m: p0T[:, k * NB0 + m * P: k * NB0 + (m + 1) * P],
                        4)
                    out_group(
                        c0, tiles, 1, D1, o1w_e, False, t1_e, M1,
                        lambda k, m: p1T[:, m * P:(m + 1) * P],
                        2)

    nc.compile()
    return nc


def _pad128(n):
    return max(P, ((n + P - 1) // P) * P)


def _install_ntff_hook():
    """Register the NTFF profile hook the agent image's antenv lacks, so
    run_bass_kernel_spmd(trace=True) can report exec_time_ns under axon."""
    import sys
    import types
    if 'antenv.axon_hooks' in sys.modules:
        return
    mod = types.ModuleType('antenv.axon_hooks')
    mod._hook = None
    mod.set_axon_ntff_profile_hook = lambda h: setattr(mod, '_hook', h)
    mod.get_axon_ntff_profile_hook = lambda: mod._hook
    sys.modules['antenv.axon_hooks'] = mod
    import antenv
    antenv.axon_hooks = mod
    from trn_agent_boot.trn_boot import _ntff_profile_via_ctypes
    mod._hook = _ntff_profile_via_ctypes('/opt/axon/libaxon_pjrt.so')
    import concourse.bass_utils as bu
    bu.upload_artifacts = lambda tmpdir: f"local:{tmpdir}"


def _run_spmd(nc, in_maps, profile):
    from concourse.bass_utils import run_bass_kernel_spmd
    kwargs = {}
    if profile:
        kwargs["trace"] = True
    return run_bass_kernel_spmd(nc, in_maps, core_ids=list(range(N_CORES)),
                                **kwargs)


def _interleave_k(a_t, kt):
    """[kt*P, F] -> [P, kt*F] with X[p, k*F + j] = a_t[k*P + p, j]."""
    ktp, F = a_t.shape
    assert ktp == kt * P
    return np.ascontiguousarray(
        a_t.reshape(kt, P, F).transpose(1, 0, 2).reshape(P, kt * F))


def _interleave_groups(a_t, kt, groups):
    """Group-blocked interleave: for each (c0, tiles) with width W, block
    [P, kt*W] with X[p, k*W + j] = a_t[k*P + p, c0 + j]; blocks concat."""
    A = a_t.reshape(kt, P, a_t.shape[1])
    blocks = []
    for c0, tiles in groups:
        W = sum(tiles)
        blocks.append(A[:, :, c0:c0 + W].transpose(1, 0, 2).reshape(P, kt * W))
    return np.ascontiguousarray(np.concatenate(blocks, axis=1))


def kernel(input, target, head_w, proj0_w, out0_w, proj1_w, out1_w):
    c0_, c1_, c2_ = CUTOFF
    h = np.ascontiguousarray(np.asarray(input, dtype=np.float32)).reshape(-1, D)
    t = np.asarray(target, dtype=np.int32).reshape(-1)
    N = h.shape[0]
    assert N % N_CORES == 0
    TOK = N // N_PAIRS          # tokens per core pair

    m0 = (t >= c0_) & (t < c1_)
    m1 = (t >= c1_) & (t < c2_)

    # int32 retarget outputs (pure indexing, negligible)
    new_t_head = np.where(m0, c0_, np.where(m1, c0_ + 1, t)).astype(np.int32)
    new_t0 = np.where(m0, t - c0_, 0).astype(np.int32)
    new_t1 = np.where(m1, t - c1_, 0).astype(np.int32)

    # per-pair routing of band rows
    idx0 = [np.nonzero(m0[p * TOK:(p + 1) * TOK])[0] for p in range(N_PAIRS)]
    idx1 = [np.nonzero(m1[p * TOK:(p + 1) * TOK])[0] for p in range(N_PAIRS)]
    NB0 = _pad128(max(len(i) for i in idx0))
    NB1 = _pad128(max(len(i) for i in idx1))

    key = (TOK, NB0, NB1)
    if key not in _graph_cache:
        _graph_cache[key] = _build_graph(TOK, NB0, NB1)
    nc = _graph_cache[key]

    bf = lambda a: a.astype(BF16)

    hw_t = np.asarray(head_w, np.float32).T      # [D, VH]
    o0_t = np.asarray(out0_w, np.float32).T      # [D0, VT]
    o1_t = np.asarray(out1_w, np.float32).T      # [D1, VT]
    halves = []
    for j in range(2):
        halves.append({
            "head_wI": bf(_interleave_groups(
                np.ascontiguousarray(hw_t[:, j * VHH:(j + 1) * VHH]),
                KT, HEAD_GROUPS)),
            "out0_wI": bf(_interleave_groups(
                np.ascontiguousarray(o0_t[:, j * VTH:(j + 1) * VTH]),
                2, TAIL_GROUPS)),
            "out1_wT": bf(np.ascontiguousarray(o1_t[:, j * VTH:(j + 1) * VTH])),
        })
    w_common = {
        "proj0_wI": bf(_interleave_k(np.ascontiguousarray(
            np.asarray(proj0_w, np.float32).T), KT)),
        "proj1_wI": bf(_interleave_k(np.ascontiguousarray(
            np.asarray(proj1_w, np.float32).T), KT)),
    }

    in_maps = []
    for p in range(N_PAIRS):
        hc = h[p * TOK:(p + 1) * TOK]
        h0 = np.zeros((NB0, D), np.float32)
        h0[:len(idx0[p])] = hc[idx0[p]]
        h1 = np.zeros((NB1, D), np.float32)
        h1[:len(idx1[p])] = hc[idx1[p]]
        acts = {
            "hT": bf(_interleave_k(np.ascontiguousarray(hc.T), KT)),
            "h0T": bf(_interleave_k(np.ascontiguousarray(h0.T), KT)),
            "h1T": bf(_interleave_k(np.ascontiguousarray(h1.T), KT)),
        }
        for j in range(2):
            in_maps.append({**acts, **w_common, **halves[j]})

    profile = bool(os.environ.get("BASS_KERNEL_PROFILE"))
    if profile:
        try:
            _install_ntff_hook()
        except Exception:
            pass
    res = _run_spmd(nc, in_maps, profile)
    if profile and res.exec_time_ns is not None:
        print(f"HW exec time: {res.exec_time_ns} ns")
        kernel.last_exec_time_ns = res.exec_time_ns

    head_out = np.empty((N, VH), np.float32)
    tail0 = np.zeros((N, VT), np.float32)
    tail1 = np.zeros((N, VT), np.float32)
    for p in range(N_PAIRS):
        base = p * TOK
        for j in range(2):
            r = res.results[2 * p + j]
            head_out[base:base + TOK, j * VHH:(j + 1) * VHH] = \
                r["head_o"].astype(np.float32)
            if len(idx0[p]):
                tail0[base + idx0[p], j * VTH:(j + 1) * VTH] = \
                    r["t0_o"][:len(idx0[p])].astype(np.float32)
            if len(idx1[p]):
                tail1[base + idx1[p], j * VTH:(j + 1) * VTH] = \
                    r["t1_o"][:len(idx1[p])].astype(np.float32)

    return (head_out, tail0, tail1, new_t_head, new_t0, new_t1)
